# revision 1
# baseline (speedup 1.0000x reference)
"""Multi-head causal attention (B=4,T=2048,C=1024,H=16,D=64) on 8 TRN2 NeuronCores.

Sharding: no collectives. Core c handles batch b=c//2 and a causally-balanced
set of four 256-query chunks (half=c%2): half0 -> chunks [7,5,2,0], half1 ->
[6,4,3,1]. Every core runs the same program with padded per-slot key-tile
counts [16,12,8,4]; per-core differences (real counts / causal diagonals) are
expressed purely through per-core input data (mask tiles). K/V projections are
computed for the full sequence on both cores of a batch (duplication instead
of cross-core communication, which is far slower on this part).

Pipeline per core (one Bass/Tile program):
  B-stage: qT = (x @ Wq)^T for the core's 1024 query columns -> SBUF resident.
  A-stage: kT = (x @ Wk)^T -> DRAM scratch; v = x @ Wv (+ones col) -> DRAM.
  C-stage: per head-pair, per key tile j: scores = kT_j^T @ qT (fp32r,
           2 heads packed via PE row groups, separate PSUM banks), exp on ACT
           (scale folded in), causal/pad masks multiplied on the one slot that
           needs them, wei @ [v|1] accumulated in PSUM ([65,256] per head:
           row 64 = sumexp). Normalize with DVE fast reciprocal + GPSIMD
           partition broadcast.
  proj:    out = attn^T-layout tiles @ Wp (+bias), written per chunk.
All matmuls run as float32r with moving free dim >= 256 (full PE rate).
"""

import numpy as np

import concourse.bass as bass
import concourse.tile as tile
from concourse import bacc, library_config, mybir
from concourse.bass_utils import run_bass_kernel_spmd

B, T, C = 4, 2048, 1024
H, D = 16, 64
P = 128            # key tile size
QC = 256           # query chunk size
NP = 8             # head pairs
PN = [16, 12, 8, 4]                    # padded per-slot key-tile counts
CHUNKS = [[7, 5, 2, 0], [6, 4, 3, 1]]  # chunk ids per half, slot order
F32 = mybir.dt.float32
F32R = mybir.dt.float32r
EXP = mybir.ActivationFunctionType.Exp
SCALE = float(C) ** -0.5


def r(ap):
    """fp32 AP -> fp32r view for full-rate PE matmuls."""
    return ap.bitcast(F32R)


def build_kernel(nc: bass.Bass):
    xT = nc.dram_tensor("xT", [C, T], F32, kind="ExternalInput").ap()
    xq = nc.dram_tensor("xq", [C, 4 * QC], F32, kind="ExternalInput").ap()
    wq2 = nc.dram_tensor("wq2", [C, C], F32, kind="ExternalInput").ap()
    wk2 = nc.dram_tensor("wk2", [C, C], F32, kind="ExternalInput").ap()
    wv2 = nc.dram_tensor("wv2", [C, C], F32, kind="ExternalInput").ap()
    wp = nc.dram_tensor("wp", [C, C], F32, kind="ExternalInput").ap()
    bias = nc.dram_tensor("bias", [1, C], F32, kind="ExternalInput").ap()
    masks = nc.dram_tensor("masks", [16, P, QC], F32, kind="ExternalInput").ap()
    eye = nc.dram_tensor("eye", [P, P], F32, kind="ExternalInput").ap()
    out = nc.dram_tensor("out", [4, QC, C], F32, kind="ExternalOutput").ap()

    kT_d = nc.dram_tensor("kT_scratch", [C, T], F32).ap()
    v_d = nc.dram_tensor("v_scratch", [16, P, 16 * 65], F32).ap()

    with tile.TileContext(nc) as tc:
        nc.gpsimd.load_library(library_config.attn)
        with tc.tile_pool(name="const", bufs=1) as cpool:
            qT_sb = cpool.tile([P, NP * 1024], F32R)
            attn_sb = cpool.tile([P, NP * 1024], F32R)
            masks_sb = cpool.tile([P, 16 * QC], F32R)
            eye_sb = cpool.tile([P, P], F32R)

            # ---------------- B then A projections ----------------
            with (
                tc.tile_pool(name="ps_mm", bufs=2, space="PSUM") as ps_mm,
                tc.tile_pool(name="xt", bufs=8) as xtpool,
                tc.tile_pool(name="stage", bufs=2) as stpool,
                tc.tile_pool(name="vstage", bufs=1) as vstpool,
            ):
                with tc.tile_pool(name="wqp", bufs=1) as wqpool:
                    wq_sb = wqpool.tile([P, 8 * 1024], F32R, tag="wq")
                    for g in range(8):
                        nc.sync.dma_start(
                            wq_sb[:, g * 1024:(g + 1) * 1024],
                            wq2[g * P:(g + 1) * P, :].bitcast(F32R),
                        )
                    for k in range(4):
                        xqg = [
                            xtpool.tile([P, QC], F32R, tag="xt0", name=f"xq{g}")
                            for g in range(8)
                        ]
                        for g in range(8):
                            nc.sync.dma_start(
                                xqg[g][:],
                                xq[g * P:(g + 1) * P,
                                   k * QC:(k + 1) * QC].bitcast(F32R),
                            )
                        for p in range(NP):
                            qps = ps_mm.tile([P, QC], F32, tag="mm")
                            for g in range(8):
                                nc.tensor.matmul(
                                    qps[:],
                                    r(wq_sb[:, g * 1024 + (2 * p) * 64:][:, :128]),
                                    xqg[g][:],
                                    start=(g == 0), stop=(g == 7),
                                )
                            nc.scalar.copy(
                                qT_sb[:, p * 1024 + k * QC:][:, :QC], qps[:]
                            )

                with tc.tile_pool(name="wkvp", bufs=1) as wkvpool:
                    wk_sb = wkvpool.tile([P, 8 * 1024], F32R, tag="wk")
                    wv_sb = wkvpool.tile([P, 8 * 1024], F32R, tag="wv")
                    for g in range(8):
                        gs = slice(g * 1024, (g + 1) * 1024)
                        nc.sync.dma_start(
                            wk_sb[:, gs], wk2[g * P:(g + 1) * P, :].bitcast(F32R)
                        )
                        nc.sync.dma_start(
                            wv_sb[:, gs], wv2[g * P:(g + 1) * P, :].bitcast(F32R)
                        )
                    for tb in range(4):
                        ts_ = slice(tb * 512, (tb + 1) * 512)
                        xtg = [
                            xtpool.tile([P, 512], F32R, tag=f"xt{tb % 2}",
                                        name=f"xt{g}")
                            for g in range(8)
                        ]
                        for g in range(8):
                            nc.sync.dma_start(
                                xtg[g][:], xT[g * P:(g + 1) * P, ts_].bitcast(F32R)
                            )
                        for p in range(NP):
                            kps = ps_mm.tile([P, 512], F32, tag="mm")
                            for g in range(8):
                                nc.tensor.matmul(
                                    kps[:],
                                    r(wk_sb[:, g * 1024 + (2 * p) * 64:][:, :128]),
                                    xtg[g][:],
                                    start=(g == 0), stop=(g == 7),
                                )
                            kst = stpool.tile([P, 512], F32, tag="kst")
                            nc.vector.tensor_copy(kst[:], kps[:])
                            nc.sync.dma_start(kT_d[p * P:(p + 1) * P, ts_], kst[:])
                        for sti in range(4):
                            j = tb * 4 + sti
                            vst = vstpool.tile([P, 16 * 65], F32, tag="vst")
                            vv = vst[:].rearrange("p (h e) -> p h e", e=65)
                            nc.vector.memset(vv[:, :, 64:65], 1.0)
                            for hc in range(2):
                                vps = ps_mm.tile([P, 512], F32, tag="mm")
                                for g in range(8):
                                    nc.tensor.matmul(
                                        vps[:],
                                        r(xtg[g][:, sti * P:(sti + 1) * P]),
                                        wv_sb[:, g * 1024 + hc * 512:][:, :512],
                                        start=(g == 0), stop=(g == 7),
                                    )
                                nc.scalar.copy(
                                    vv[:, hc * 8:(hc + 1) * 8, 0:64],
                                    vps[:].rearrange("p (h d) -> p h d", d=64),
                                )
                            nc.sync.dma_start(v_d[j], vst[:])

            # ---------------- C: attention + proj ----------------
            for i in range(16):
                nc.sync.dma_start(
                    masks_sb[:, i * QC:(i + 1) * QC], masks[i].bitcast(F32R)
                )
            nc.sync.dma_start(eye_sb[:], eye[:].bitcast(F32R))
            with (
                tc.tile_pool(name="kv", bufs=2) as kvpool,
                tc.tile_pool(name="exp", bufs=3) as epool,
                tc.tile_pool(name="norm", bufs=1) as npool,
                tc.tile_pool(name="wpp", bufs=1) as wppool,
                tc.tile_pool(name="outp", bufs=3) as outpool,
                tc.tile_pool(name="ps_sc", bufs=2, space="PSUM") as ps_sc,
                tc.tile_pool(name="ps_av", bufs=2, space="PSUM") as ps_av,
                tc.tile_pool(name="ps_pj", bufs=2, space="PSUM") as ps_pj,
            ):
                wp_sb = wppool.tile([P, 8 * 1024], F32R, tag="wp")
                for g in range(8):
                    nc.sync.dma_start(
                        wp_sb[:, g * 1024:(g + 1) * 1024],
                        wp[g * P:(g + 1) * P, :].bitcast(F32R),
                    )
                bias_s = wppool.tile([1, C], F32, tag="bias1")
                nc.sync.dma_start(bias_s[:], bias[:])
                bias_bc = wppool.tile([P, C], F32, tag="biasbc")
                nc.gpsimd.partition_broadcast(bias_bc[:], bias_s[:])

                def c_run(k, p):
                    avp = ps_av.tile([65, 2 * QC], F32, tag="av",
                                     name=f"av{k}_{p}")
                    qA = qT_sb[0:64, p * 1024 + k * QC:][:, :QC]
                    qB = qT_sb[64:128, p * 1024 + k * QC:][:, :QC]
                    njc = PN[k] // 4
                    pend = None  # (e_t, v0, v1, j0) awaiting AV emission

                    def emit_av(pv):
                        e_t, v0, v1, j0 = pv
                        nc.tensor.matmul(avp[:, 0:QC], v0[:, 0:65],
                                         e_t[:, 0:QC],
                                         start=(j0 == 0), stop=False)
                        nc.tensor.matmul(avp[:, 0:QC], v1[:, 0:65],
                                         e_t[:, QC:2 * QC],
                                         start=False, stop=False)
                        nc.tensor.matmul(avp[:, QC:2 * QC], v0[:, 65:130],
                                         e_t[:, 2 * QC:3 * QC],
                                         start=False, stop=False)
                        nc.tensor.matmul(avp[:, QC:2 * QC], v1[:, 65:130],
                                         e_t[:, 3 * QC:4 * QC],
                                         start=False, stop=(j0 + 1 == PN[k] - 1))

                    for jc in range(njc):
                        ktc = kvpool.tile([P, 4 * P], F32R, tag="kt")
                        nc.sync.dma_start(
                            ktc[:],
                            kT_d[p * P:(p + 1) * P,
                                 jc * 4 * P:(jc + 1) * 4 * P].bitcast(F32R),
                        )
                        vc = kvpool.tile([P, 4 * 130], F32R, tag="vt")
                        nc.sync.dma_start(
                            vc[:].rearrange("s (j c) -> s j c", c=130),
                            v_d[4 * jc:4 * jc + 4, :,
                                2 * p * 65:(2 * p + 2) * 65]
                            .rearrange("j s c -> s j c").bitcast(F32R),
                        )
                        for u in range(2):
                            j0 = 4 * jc + 2 * u
                            masked = j0 >= PN[k] - 4
                            kt0 = ktc[:, (2 * u) * P:(2 * u + 1) * P]
                            kt1 = ktc[:, (2 * u + 1) * P:(2 * u + 2) * P]
                            v0 = vc[:, (2 * u) * 130:(2 * u + 1) * 130]
                            v1 = vc[:, (2 * u + 1) * 130:(2 * u + 2) * 130]
                            sc = ps_sc.tile([P, 4 * QC], F32, tag="sc")
                            nc.tensor.matmul(sc[:, 0:QC], r(kt0[0:64, :]), qA,
                                             start=True, stop=False,
                                             tile_position=(0, 0))
                            nc.tensor.matmul(sc[:, 2 * QC:3 * QC],
                                             r(kt0[64:128, :]), qB,
                                             start=True, stop=False,
                                             tile_position=(64, 0))
                            nc.tensor.matmul(sc[:, QC:2 * QC], r(kt1[0:64, :]),
                                             qA, start=False, stop=not masked,
                                             tile_position=(0, 0))
                            nc.tensor.matmul(sc[:, 3 * QC:4 * QC],
                                             r(kt1[64:128, :]), qB,
                                             start=False, stop=not masked,
                                             tile_position=(64, 0))
                            if masked:
                                li = (k * 4 + (j0 - (PN[k] - 4))) * QC
                                mb = masks_sb[:, li:li + 2 * QC]
                                nc.tensor.matmul(sc[:, 0:2 * QC], eye_sb[:], mb,
                                                 start=False, stop=True)
                                nc.tensor.matmul(sc[:, 2 * QC:4 * QC], eye_sb[:],
                                                 mb, start=False, stop=True)
                            e_t = epool.tile([P, 4 * QC], F32R, tag="exp")
                            nc.scalar.activation(e_t[:], sc[:], EXP, scale=SCALE)
                            if pend is not None:
                                emit_av(pend)
                            pend = (e_t, v0, v1, j0)
                    emit_av(pend)
                    rs = npool.tile([1, 2 * QC], F32, tag="rs", bufs=2)
                    nc.vector.tensor_copy(rs[:], avp[64:65, :])
                    avc = npool.tile([64, 2 * QC], F32, tag="avc", bufs=2)
                    nc.vector.tensor_copy(avc[:], avp[0:64, :])
                    rc = npool.tile([1, 2 * QC], F32, tag="rc", bufs=2)
                    nc.vector.reciprocal_approx_fast(rc[:], rs[:])
                    rb = npool.tile([64, 2 * QC], F32, tag="rb", bufs=2)
                    nc.gpsimd.partition_broadcast(rb[:], rc[:])
                    col = p * 1024 + k * QC
                    nc.vector.tensor_mul(attn_sb[0:64, col:col + QC],
                                         avc[:, 0:QC], rb[:, 0:QC])
                    nc.vector.tensor_mul(attn_sb[64:128, col:col + QC],
                                         avc[:, QC:2 * QC], rb[:, QC:2 * QC])

                def proj_unit(k, tt, oc):
                    pp = ps_pj.tile([P, 512], F32, tag="pj")
                    for g in range(NP):
                        nc.tensor.matmul(
                            pp[:],
                            r(attn_sb[:, g * 1024 + k * QC + tt * P:][:, :P]),
                            wp_sb[:, g * 1024 + oc * 512:][:, :512],
                            start=(g == 0), stop=(g == 7),
                        )
                    ot = outpool.tile([P, 512], F32, tag="ot")
                    nc.vector.tensor_add(
                        ot[:], pp[:], bias_bc[:, oc * 512:(oc + 1) * 512]
                    )
                    nc.sync.dma_start(
                        out[k, tt * P:(tt + 1) * P, oc * 512:(oc + 1) * 512],
                        ot[:],
                    )

                for k in (3, 2, 1, 0):
                    for p in range(NP):
                        c_run(k, p)
                    for tt in range(2):
                        for oc in range(2):
                            proj_unit(k, tt, oc)
    return nc


def _make_masks(half):
    chunks = CHUNKS[half]
    m = np.zeros((16, P, QC), np.float32)
    s = np.arange(P)[:, None]
    t = np.arange(QC)[None, :]
    for k in range(4):
        q = chunks[k]
        n = 2 * (q + 1)
        for l in range(4):
            j = PN[k] - 4 + l
            if j >= n:
                pat = np.full((P, QC), -1e6, np.float32)
            elif j == n - 2:
                pat = np.where(s <= t, 0.0, -1e6).astype(np.float32)
            elif j == n - 1:
                pat = np.where(s <= t - 128, 0.0, -1e6).astype(np.float32)
            else:
                pat = np.zeros((P, QC), np.float32)
            m[k * 4 + l] = pat
    return m


_CACHE = {}


def _get_nc():
    if "nc" not in _CACHE:
        nc = bacc.Bacc("TRN2", target_bir_lowering=False, debug=False)
        build_kernel(nc)
        nc.compile()
        _CACHE["nc"] = nc
    return _CACHE["nc"]


def make_in_maps(x, wq, wk, wv, w_proj, b_proj):
    x = np.ascontiguousarray(np.asarray(x, np.float32))
    wq2 = np.ascontiguousarray(np.transpose(np.asarray(wq), (1, 0, 2)).reshape(C, C))
    wk2 = np.ascontiguousarray(np.transpose(np.asarray(wk), (1, 0, 2)).reshape(C, C))
    wv2 = np.ascontiguousarray(np.transpose(np.asarray(wv), (1, 0, 2)).reshape(C, C))
    wpm = np.ascontiguousarray(np.asarray(w_proj, np.float32))
    bias = np.asarray(b_proj, np.float32).reshape(1, C)
    masks_h = [_make_masks(0), _make_masks(1)]

    in_maps = []
    for core in range(8):
        b, half = core // 2, core % 2
        xTb = np.ascontiguousarray(x[b].T)
        xqb = np.ascontiguousarray(
            np.concatenate(
                [xTb[:, q * QC:(q + 1) * QC] for q in CHUNKS[half]], axis=1
            )
        )
        in_maps.append({
            "xT": xTb, "xq": xqb,
            "wq2": wq2, "wk2": wk2, "wv2": wv2,
            "wp": wpm, "bias": bias, "masks": masks_h[half],
            "eye": np.eye(P, dtype=np.float32),
        })
    return in_maps


def assemble(results):
    full = np.zeros((B, T, C), np.float32)
    for core in range(8):
        b, half = core // 2, core % 2
        o = results[core]["out"]
        for k, q in enumerate(CHUNKS[half]):
            full[b, q * QC:(q + 1) * QC] = o[k]
    return full


def kernel(x, wq, wk, wv, w_proj, b_proj, _trace=False, _tmpdir=None):
    in_maps = make_in_maps(x, wq, wk, wv, w_proj, b_proj)
    nc = _get_nc()
    res = run_bass_kernel_spmd(
        nc, in_maps, core_ids=list(range(8)), trace=_trace, tmpdir=_tmpdir
    )
    if _trace:
        _CACHE["last_result"] = res
    return assemble(res.results)

